# revision 1
# baseline (speedup 1.0000x reference)
"""Trainium2 Bass kernel for nn_Attention_90658169684243.

Attention-LSTM decoder: 3x3 conv (512->512) over [B,512,8,32] feature maps,
26 sequential steps of {additive attention over 256 spatial positions,
2-layer LSTM}, and a linear head.

Sharding: data-parallel over batch across 8 cores (B=256 -> 32/core), all
parameters replicated. bf16 on the matmul path with fp32 PSUM accumulation;
softmax and LSTM cell math in fp32. Sigmoid is computed as
0.5*tanh(0.5x)+0.5 so the whole kernel uses one ACT table set (exp/tanh).
"""

import numpy as np
import ml_dtypes

bfnp = ml_dtypes.bfloat16

NCORES = 8
BFULL = 256
B = BFULL // NCORES   # 32 per core
C = 512
HF, WF = 8, 32
HW = HF * WF          # 256
T = 26
HS = 512
NCLS = 38
G4 = 4 * HS           # 2048

_CACHE = {}


def _build():
    import contextlib

    import concourse.bacc as bacc
    import concourse.mybir as mybir
    from concourse import tile

    dt = mybir.dt
    f32 = dt.float32
    bf = dt.bfloat16
    AF = mybir.ActivationFunctionType
    OP = mybir.AluOpType

    nc = bacc.Bacc(None)

    def din(name, shape, dtype=bf):
        return nc.dram_tensor(name, shape, dtype, kind="ExternalInput")

    fm_ci = din("fm_ci", [4, 128, B, HF, WF])
    w9d = din("w9d", [3, 3, 4, 128, C])
    conv_bT = din("conv_bT", [4, 128, 1], f32)
    bhmT = din("bhmT", [4, 128, B])
    i2hT = din("i2hT", [4, 128, HS])
    bh_bias = din("bh_bias", [B, HS], f32)
    h0T = din("h0T", [4, 128, B])
    c0 = din("c0", [B, HS], f32)
    onehT = din("onehT", [NCLS + 1, T, B])
    h2hTd = din("h2hTd", [4, 128, HS])
    w1x1Td = din("w1x1Td", [4, 128, HS])
    b1x1Td = din("b1x1Td", [4, 128, 1], f32)
    hlinTd = din("hlinTd", [4, 128, HS])
    hlin_brow = din("hlin_brow", [1, HS])
    wih1Td = din("wih1Td", [4, 128, G4])
    tail1Td = din("tail1Td", [NCLS + 1, G4])
    whh1Td = din("whh1Td", [4, 128, G4])
    wih2Td = din("wih2Td", [4, 128, G4])
    whh2Td = din("whh2Td", [4, 128, G4])
    b2row = din("b2row", [1, G4])
    wsc_repd = din("wsc_repd", [4, 128, B])
    gen_wTd = din("gen_wTd", [4, 128, NCLS])
    gen_bTd = din("gen_bTd", [NCLS, 1], f32)
    identd = din("identd", [128, 128])

    probsT = nc.dram_tensor("probsT", [NCLS, T * B], f32, kind="ExternalOutput")

    with tile.TileContext(nc) as tc:
        stack = contextlib.ExitStack()
        const = stack.enter_context(tc.tile_pool(name="const", bufs=1))
        big = stack.enter_context(tc.tile_pool(name="big", bufs=1))
        state = stack.enter_context(tc.tile_pool(name="state", bufs=2))

        fmh = [big.tile([128, B, HW], bf, tag=f"fmh{i}", name=f"fmh{i}")
               for i in range(4)]
        fmhT = [big.tile([128, B, C], bf, tag=f"fmhT{i}", name=f"fmhT{i}")
                for i in range(2)]

        def cload(name, src, shape, dtype=bf, pool=None):
            t = (pool or const).tile(shape, dtype, tag=name, name=name)
            nc.sync.dma_start(t[:], src)
            return t

        ones = const.tile([1, B], bf, tag="ones", name="ones")
        nc.vector.memset(ones[:], 1.0)
        ones128 = const.tile([128, B], bf, tag="ones128", name="ones128")
        nc.vector.memset(ones128[:], 1.0)
        bh_plus = const.tile([B, HS], f32, tag="bh_plus", name="bh_plus")

        # ---------------- phase 1: conv (+ bh_proj) ----------------
        with (
            tc.tile_pool(name="cpad", bufs=1) as cpad,
            tc.tile_pool(name="cw", bufs=1) as cw,
            tc.tile_pool(name="cps", bufs=4, space="PSUM") as cps,
            tc.tile_pool(name="cpt", bufs=4, space="PSUM") as cpt,
        ):
            ident = cw.tile([128, 128], bf, tag="ident", name="ident")
            nc.sync.dma_start(ident[:], identd[:])
            conv_b = []
            for k in range(4):
                cb = cw.tile([128, 1], f32, tag=f"conv_b{k}", name=f"conv_b{k}")
                nc.sync.dma_start(cb[:], conv_bT[k])
                conv_b.append(cb)
            w9 = [[[cw.tile([128, C], bf, tag=f"w9_{kh}{kw}{ci}",
                            name=f"w9_{kh}{kw}{ci}")
                    for ci in range(4)] for kw in range(3)] for kh in range(3)]
            for kh in range(3):
                for kw in range(3):
                    for ci in range(4):
                        nc.gpsimd.dma_start(w9[kh][kw][ci][:], w9d[kh, kw, ci])

            BC = 2  # batch chunk for conv
            for bc in range(B // BC):
                b0 = bc * BC
                pads = []
                for ci in range(4):
                    pad = cpad.tile([128, BC, HF + 2, WF + 2], bf,
                                    tag=f"pad{ci}", name=f"pad{ci}")
                    nc.vector.memset(pad[:, :, 0, :], 0.0)
                    nc.vector.memset(pad[:, :, HF + 1, :], 0.0)
                    nc.vector.memset(pad[:, :, 1:HF + 1, 0], 0.0)
                    nc.vector.memset(pad[:, :, 1:HF + 1, WF + 1], 0.0)
                    for b in range(BC):
                        nc.gpsimd.dma_start(pad[:, b, 1:HF + 1, 1:WF + 1],
                                            fm_ci[ci, :, b0 + b])
                    pads.append(pad)
                for co in range(4):
                    ps = cps.tile([128, BC, HW], f32, tag="pscv", name="pscv")
                    idx = 0
                    for kh in range(3):
                        for kw in range(3):
                            for ci in range(4):
                                nc.tensor.matmul(
                                    ps[:],
                                    w9[kh][kw][ci][:, co * 128:(co + 1) * 128],
                                    pads[ci][:, :, kh:kh + HF, kw:kw + WF],
                                    start=(idx == 0), stop=(idx == 35))
                                idx += 1
                    for b in range(BC):
                        nc.vector.tensor_scalar_add(
                            fmh[co][:, b0 + b, :], ps[:, b, :],
                            conv_b[co][:, 0:1])
                    for b in range(BC):
                        for hh in range(2):
                            pt = cpt.tile([128, 128], bf, tag="pst", name="pst")
                            nc.tensor.transpose(
                                pt[:],
                                fmh[co][:, b0 + b, hh * 128:(hh + 1) * 128],
                                ident[:])
                            nc.vector.tensor_copy(
                                fmhT[hh][:, b0 + b, co * 128:(co + 1) * 128],
                                pt[:])

        # ---- bh_proj_plus = mean_t(batch_H) @ i2h^T + h2h_b (once) ----
        with (
            tc.tile_pool(name="pre", bufs=1) as pre,
            tc.tile_pool(name="prep", bufs=1, space="PSUM") as prep,
        ):
            i2h = [pre.tile([128, HS], bf, tag=f"i2h{k}", name=f"i2h{k}")
                   for k in range(4)]
            bhm = [pre.tile([128, B], bf, tag=f"bhm{k}", name=f"bhm{k}")
                   for k in range(4)]
            bh_b = pre.tile([B, HS], f32, tag="bh_b", name="bh_b")
            nc.sync.dma_start(bh_b[:], bh_bias[:])
            for k in range(4):
                nc.gpsimd.dma_start(i2h[k][:], i2hT[k])
                nc.gpsimd.dma_start(bhm[k][:], bhmT[k])
            ps_bh = prep.tile([B, HS], f32, tag="psbh", name="psbh")
            for k in range(4):
                nc.tensor.matmul(ps_bh[:], bhm[k][:], i2h[k][:],
                                 start=(k == 0), stop=(k == 3))
            nc.vector.tensor_tensor(bh_plus[:], ps_bh[:], bh_b[:], OP.add)

        # ---------------- phase 2: 26-step scan ----------------
        wconst = stack.enter_context(tc.tile_pool(name="wconst", bufs=1))
        h2hT = [cload(f"h2hT{k}", h2hTd[k], [128, HS], pool=wconst) for k in range(4)]
        w1x1T = [cload(f"w1x1T{k}", w1x1Td[k], [128, HS], pool=wconst) for k in range(4)]
        b1x1T = [cload(f"b1x1T{k}", b1x1Td[k], [128, 1], f32, pool=wconst) for k in range(4)]
        hlinT = [cload(f"hlinT{k}", hlinTd[k], [128, HS], pool=wconst) for k in range(4)]
        h1T = [cload(f"h1T_{k}", h0T[k], [128, B], pool=wconst) for k in range(4)]
        h2T = [cload(f"h2T_{k}", h0T[k], [128, B], pool=wconst) for k in range(4)]
        c1 = cload("c1", c0[:], [B, HS], f32, pool=wconst)
        c2 = cload("c2", c0[:], [B, HS], f32, pool=wconst)
        hlin_b = cload("hlin_b", hlin_brow[:], [1, HS], pool=wconst)
        tail1T = cload("tail1T", tail1Td[:], [NCLS + 1, G4], pool=wconst)
        b2r = cload("b2r", b2row[:], [1, G4], pool=wconst)
        wsc_rep = [cload(f"wsc_rep{k}", wsc_repd[k], [128, B], pool=wconst) for k in range(4)]
        gen_wT = [cload(f"gen_wT{k}", gen_wTd[k], [128, NCLS], pool=wconst) for k in range(4)]
        gen_bT = cload("gen_bT", gen_bTd[:], [NCLS, 1], f32, pool=wconst)
        oneh = cload("oneh", onehT[:], [NCLS + 1, T, B], pool=wconst)
        h2all = [big.tile([128, T * B], bf, tag=f"h2all{i}", name=f"h2all{i}")
                 for i in range(4)]
        sb = stack.enter_context(tc.tile_pool(name="sb", bufs=2))
        sb1 = stack.enter_context(tc.tile_pool(name="sb1", bufs=1))
        tp = stack.enter_context(tc.tile_pool(name="tp", bufs=2))
        ws = stack.enter_context(tc.tile_pool(name="ws", bufs=2))
        mm = stack.enter_context(tc.tile_pool(name="mm", bufs=2, space="PSUM"))

        for t in range(T):
            # ---- v = h2 @ h2h_w^T + (bh_proj + h2h_b) ----
            ps_v = mm.tile([B, HS], f32, tag="mm", name="mm")
            for k in range(4):
                nc.tensor.matmul(ps_v[:], h2T[k][:, :], h2hT[k][:],
                                 start=(k == 0), stop=(k == 3))
            v_bf = sb1.tile([B, HS], bf, tag="vb", name="v_bf")
            nc.vector.tensor_tensor(v_bf[:], ps_v[:], bh_plus[:], OP.add)
            vT = [sb.tile([128, B], bf, tag=f"vT{k}", name=f"vT{k}")
                  for k in range(4)]
            t32(nc, vT, v_bf[:], HS)

            # ---- q = v @ w1x1^T (bias folded into attention add) ----
            ps_q = mm.tile([B, HS], f32, tag="mm", name="mm")
            for k in range(4):
                nc.tensor.matmul(ps_q[:], vT[k][:], w1x1T[k][:],
                                 start=(k == 0), stop=(k == 3))
            q_sb = sb1.tile([B, HS], f32, tag="th4", name="q_sb")
            nc.vector.tensor_copy(q_sb[:], ps_q[:])
            qT = [sb.tile([128, B], f32, tag=f"qT{k}", name=f"qT{k}")
                  for k in range(4)]
            t32(nc, qT, q_sb[:], HS)

            # ---- e[b, hw] = sum_c wsc_c * tanh(fmh + q + b1x1) ----
            # lhsT = w_score replicated over 32 cols -> all PSUM rows
            # identical; row bb at free block i is e for batch bb, so the
            # extraction copy stays on one partition.
            e_sb = sb1.tile([B, HW], f32, tag="e_sb", name="e_sb")
            for g in range(8):        # groups of 4 batch rows
                gb = g * 4
                ps_e = mm.tile([B, 4, HW], f32, tag="mm", name="mm")
                for ct in range(4):
                    for nb in range(2):
                        tt = tp.tile([128, 2, HW], bf, tag="t", name="t")
                        for i2 in range(2):
                            i = nb * 2 + i2
                            nc.vector.tensor_scalar(
                                tt[:, i2, :], fmh[ct][:, gb + i, :],
                                qT[ct][:, gb + i:gb + i + 1],
                                b1x1T[ct][:, 0:1], OP.add, OP.add)
                        nc.scalar.activation(tt[:], tt[:], AF.Tanh)
                        nc.tensor.matmul(
                            ps_e[:, nb * 2:nb * 2 + 2, :],
                            wsc_rep[ct][:],
                            tt[:],
                            start=(ct == 0), stop=(ct == 3))
                # all PSUM rows identical: stage row 0 to SBUF, then DMA
                # scatters the four b-rows to their partitions.
                # HW quirk: ACT copies with multi-dim free APs from PSUM
                # corrupt the 2nd block, and 1->N-partition scatter DMAs with
                # multi-dim source APs misplace data -> do both per row.
                for half in range(2):
                    es = sb.tile([1, 2, HW], f32, tag="es", name="es")
                    for i2 in range(2):
                        r = half * 2 + i2
                        nc.scalar.copy(es[:, i2, :], ps_e[0:1, r, :])
                        nc.scalar.dma_start(e_sb[gb + r:gb + r + 1, :],
                                            es[0:1, i2, :])

            # ---- softmax over hw (score_b dropped: shift-invariant) ----
            neg_m = sb.tile([B, 1], f32, tag="neg_m", name="neg_m")
            nc.vector.tensor_reduce(neg_m[:], e_sb[:], mybir.AxisListType.X,
                                    OP.max, negate=True)
            expz = sb.tile([B, HW], f32, tag="es", name="expz")
            nc.scalar.activation(expz[:], e_sb[:], AF.Exp, bias=neg_m[:, 0:1])
            zsum = sb.tile([B, 1], f32, tag="zsum", name="zsum")
            nc.vector.tensor_reduce(zsum[:], expz[:], mybir.AxisListType.X,
                                    OP.add)
            rz = sb.tile([B, 1], f32, tag="rz", name="rz")
            nc.vector.reciprocal(rz[:], zsum[:])
            alpha = sb1.tile([B, HW], f32, tag="e_sb", name="alpha")
            nc.vector.tensor_scalar_mul(alpha[:], expz[:], rz[:, 0:1])
            alphaT = [sb.tile([128, B], f32, tag=f"alphaT{k}", name=f"alphaT{k}")
                      for k in range(2)]
            t32(nc, alphaT, alpha[:], HW)

            # ---- context[b, c] = sum_hw alpha * fmh ----
            # lhsT = full alphaT [128, 32]: PSUM row b' uses alpha_b'; the
            # diagonal row b' = bb at free block i is the true context.
            ctx_bf = sb1.tile([B, HS], bf, tag="vb", name="ctx_bf")
            for g in range(8):        # groups of 4 batch rows
                ps_c = mm.tile([B, 4, HS], f32, tag="mm", name="mm")
                for i in range(4):
                    bb = g * 4 + i
                    for kt in range(2):
                        # replicate alphaT column bb across 32 lhsT columns
                        # so every PSUM row holds context for batch bb
                        arep = sb.tile([128, B], bf, tag=f"arep{kt}",
                                       name=f"arep{kt}")
                        nc.vector.tensor_scalar(
                            arep[:], ones128[:],
                            alphaT[kt][:, bb:bb + 1], None, OP.mult)
                        nc.tensor.matmul(
                            ps_c[:, i, :],
                            arep[:],
                            fmhT[kt][:, bb, :],
                            start=(kt == 0), stop=(kt == 1))
                for half in range(2):
                    cs = sb.tile([1, 2, HS], bf, tag="cs", name="cs")
                    for i2 in range(2):
                        r = half * 2 + i2
                        nc.scalar.copy(cs[:, i2, :], ps_c[0:1, r, :])
                        nc.scalar.dma_start(
                            ctx_bf[g * 4 + r:g * 4 + r + 1, :],
                            cs[0:1, i2, :])
            xT = [sb.tile([128, B], bf, tag=f"xT{k}", name=f"xT{k}")
                  for k in range(4)]
            t32(nc, xT, ctx_bf[:], HS)

            # ---- LSTM 1 gates (k-outer so streamed weights die fast) ----
            ps_g = mm.tile([B, G4], f32, tag="mm", name="mm")
            for k in range(4):
                w = ws.tile([128, G4], bf, tag="ws", name="ws")
                nc.gpsimd.dma_start(w[:], wih1Td[k])
                for nb in range(4):
                    nc.tensor.matmul(ps_g[:, nb * HS:(nb + 1) * HS], xT[k][:],
                                     w[:, nb * HS:(nb + 1) * HS],
                                     start=(k == 0), stop=False)
            for nb in range(4):
                nc.tensor.matmul(ps_g[:, nb * HS:(nb + 1) * HS],
                                 oneh[:, t, :], tail1T[:, nb * HS:(nb + 1) * HS],
                                 start=False, stop=False)
            for k in range(4):
                w = ws.tile([128, G4], bf, tag="ws", name="ws")
                nc.gpsimd.dma_start(w[:], whh1Td[k])
                for nb in range(4):
                    nc.tensor.matmul(ps_g[:, nb * HS:(nb + 1) * HS], h1T[k][:],
                                     w[:, nb * HS:(nb + 1) * HS],
                                     start=False, stop=(k == 3))

            def lstm_cell(ps, c_prev, tag):
                # th4 slices: 0=i, 1=f, 2=g, 3=o
                th4 = sb1.tile([B, 4, HS], f32, tag="th4", name="th4")
                nc.scalar.activation(th4[:, 0, :], ps[:, 0:HS], AF.Tanh, scale=0.5)
                nc.scalar.activation(th4[:, 1, :], ps[:, HS:2 * HS], AF.Tanh,
                                     scale=0.5)
                nc.scalar.activation(th4[:, 2, :], ps[:, 2 * HS:3 * HS], AF.Tanh)
                nc.scalar.activation(th4[:, 3, :], ps[:, 3 * HS:4 * HS], AF.Tanh,
                                     scale=0.5)
                for sl in (0, 1, 3):  # sigmoid = 0.5*tanh(0.5x) + 0.5
                    nc.vector.tensor_scalar(th4[:, sl, :], th4[:, sl, :],
                                            0.5, 0.5, OP.mult, OP.add)
                nc.vector.tensor_tensor(th4[:, 1, :], th4[:, 1, :], c_prev[:],
                                        OP.mult)
                nc.vector.tensor_tensor(th4[:, 0, :], th4[:, 0, :], th4[:, 2, :],
                                        OP.mult)
                c_new = state.tile([B, HS], f32, tag=f"c{tag}", name=f"c{tag}")
                nc.vector.tensor_tensor(c_new[:], th4[:, 1, :], th4[:, 0, :],
                                        OP.add)
                nc.scalar.activation(th4[:, 2, :], c_new[:], AF.Tanh)
                h_bf = sb.tile([B, HS], bf, tag="hbf", name=f"hbf{tag}")
                nc.vector.tensor_tensor(h_bf[:], th4[:, 3, :], th4[:, 2, :],
                                        OP.mult)
                return c_new, h_bf

            c1, h1_bf = lstm_cell(ps_g, c1, "1")
            h1T = [state.tile([128, B], bf, tag=f"h1T{k}", name=f"h1T{k}")
                   for k in range(4)]
            t32(nc, h1T, h1_bf[:], HS)

            # ---- cur = h1 @ hlin_w^T + hlin_b ----
            ps_h = mm.tile([B, HS], f32, tag="mm", name="mm")
            for k in range(4):
                nc.tensor.matmul(ps_h[:], h1T[k][:], hlinT[k][:],
                                 start=(k == 0), stop=False)
            nc.tensor.matmul(ps_h[:], ones[:], hlin_b[:], start=False, stop=True)
            cur_bf = sb1.tile([B, HS], bf, tag="vb", name="cur_bf")
            nc.scalar.copy(cur_bf[:], ps_h[:])
            curT = [sb.tile([128, B], bf, tag=f"curT{k}", name=f"curT{k}")
                    for k in range(4)]
            t32(nc, curT, cur_bf[:], HS)

            # ---- LSTM 2 gates ----
            ps_g2 = mm.tile([B, G4], f32, tag="mm", name="mm")
            for k in range(4):
                w = ws.tile([128, G4], bf, tag="ws", name="ws")
                nc.gpsimd.dma_start(w[:], wih2Td[k])
                for nb in range(4):
                    nc.tensor.matmul(ps_g2[:, nb * HS:(nb + 1) * HS], curT[k][:],
                                     w[:, nb * HS:(nb + 1) * HS],
                                     start=(k == 0), stop=False)
            for k in range(4):
                w = ws.tile([128, G4], bf, tag="ws", name="ws")
                nc.gpsimd.dma_start(w[:], whh2Td[k])
                for nb in range(4):
                    nc.tensor.matmul(ps_g2[:, nb * HS:(nb + 1) * HS], h2T[k][:],
                                     w[:, nb * HS:(nb + 1) * HS],
                                     start=False, stop=False)
            for nb in range(4):
                nc.tensor.matmul(ps_g2[:, nb * HS:(nb + 1) * HS], ones[:],
                                 b2r[:, nb * HS:(nb + 1) * HS],
                                 start=False, stop=True)

            c2, h2_bf = lstm_cell(ps_g2, c2, "2")
            h2T = [h2all[k][:, t * B:(t + 1) * B] for k in range(4)]
            t32(nc, h2T, h2_bf[:], HS)

        # ---------------- phase 3: probs = h2_all @ gen_w^T + gen_b ----------------
        out_sb = sb1.tile([NCLS, T * B], f32, tag="th4", name="out_sb")
        for n0, n1 in ((0, 512), (512, T * B)):
            ps_p = mm.tile([NCLS, n1 - n0], f32, tag="mm", name="mm")
            for k in range(4):
                nc.tensor.matmul(ps_p[:], gen_wT[k][:], h2all[k][:, n0:n1],
                                 start=(k == 0), stop=(k == 3))
            nc.scalar.activation(out_sb[:, n0:n1], ps_p[:], AF.Identity,
                                 bias=gen_bT[:, 0:1])
        nc.sync.dma_start(probsT[:], out_sb[:])

        stack.close()

    nc.compile()
    return nc


def t32(nc, dst_tiles, src_ap, ncols):
    """Transpose src [32, ncols] into tiles of [128, 32] via DVE 32x32 block
    transposes: block j of src lands at dst_tiles[j // 4] rows (j % 4)*32."""
    for j in range(ncols // 32):
        kt, r = j // 4, (j % 4) * 32
        nc.vector.transpose(dst_tiles[kt][r:r + 32, :],
                            src_ap[:, j * 32:(j + 1) * 32])


def _prep_core(inputs, c):
    """Per-core input map (host-side reshape/transpose/cast only)."""
    f32 = np.float32
    sl = slice(c * B, (c + 1) * B)
    fm = np.asarray(inputs["feature_map"], f32)[sl]
    fm_ci = np.ascontiguousarray(fm.transpose(1, 0, 2, 3)).reshape(
        4, 128, B, HF, WF)

    def bfa(x):
        return np.ascontiguousarray(x).astype(bfnp)

    w9 = np.asarray(inputs["conv_m2h_w"], f32).transpose(2, 3, 1, 0)
    w9d = w9.reshape(3, 3, 4, 128, C)

    bhm = np.asarray(inputs["batch_H"], f32)[sl].mean(axis=1)
    bhmT = bhm.T.reshape(4, 128, B)

    hh = np.asarray(inputs["hidden_h"], f32)
    hc = np.asarray(inputs["hidden_c"], f32)
    h0 = ((hh[0] + hh[1]) * 0.5)[sl]
    c0 = ((hc[0] + hc[1]) * 0.5)[sl]

    text = np.asarray(inputs["text"])[sl]
    onehT = np.zeros((NCLS + 1, T, B), f32)
    for b in range(B):
        for t in range(T):
            onehT[int(text[b, t]), t, b] = 1.0
    onehT[NCLS] = 1.0

    b1 = np.asarray(inputs["rnn1_b_ih"], f32) + np.asarray(inputs["rnn1_b_hh"], f32)
    b2 = np.asarray(inputs["rnn2_b_ih"], f32) + np.asarray(inputs["rnn2_b_hh"], f32)
    wih1T = np.asarray(inputs["rnn1_w_ih"], f32).T
    tail1T = np.concatenate([wih1T[512:550], b1[None]], axis=0)
    wsc = np.asarray(inputs["score_w"], f32)[0, :, 0, 0]

    return {
        "fm_ci": bfa(fm_ci),
        "w9d": bfa(w9d),
        "conv_bT": np.asarray(inputs["conv_m2h_b"], f32).reshape(4, 128, 1),
        "bhmT": bfa(bhmT),
        "i2hT": bfa(np.asarray(inputs["i2h_w"], f32).T.reshape(4, 128, HS)),
        "bh_bias": np.tile(np.asarray(inputs["h2h_b"], f32)[None], (B, 1)),
        "h0T": bfa(h0.T.reshape(4, 128, B)),
        "c0": np.ascontiguousarray(c0),
        "onehT": bfa(onehT),
        "h2hTd": bfa(np.asarray(inputs["h2h_w"], f32).T.reshape(4, 128, HS)),
        "w1x1Td": bfa(np.asarray(inputs["conv_h2h_w"], f32)[:, :, 0, 0].T
                      .reshape(4, 128, HS)),
        "b1x1Td": np.asarray(inputs["conv_h2h_b"], f32).reshape(4, 128, 1),
        "hlinTd": bfa(np.asarray(inputs["hlin_w"], f32).T.reshape(4, 128, HS)),
        "hlin_brow": bfa(np.asarray(inputs["hlin_b"], f32)[None]),
        "wih1Td": bfa(wih1T[:512].reshape(4, 128, G4)),
        "tail1Td": bfa(tail1T),
        "whh1Td": bfa(np.asarray(inputs["rnn1_w_hh"], f32).T.reshape(4, 128, G4)),
        "wih2Td": bfa(np.asarray(inputs["rnn2_w_ih"], f32).T.reshape(4, 128, G4)),
        "whh2Td": bfa(np.asarray(inputs["rnn2_w_hh"], f32).T.reshape(4, 128, G4)),
        "b2row": bfa(b2[None]),
        "wsc_repd": bfa(np.tile(wsc.reshape(4, 128, 1), (1, 1, B))),
        "gen_wTd": bfa(np.asarray(inputs["gen_w"], f32).T.reshape(4, 128, NCLS)),
        "gen_bTd": np.asarray(inputs["gen_b"], f32).reshape(NCLS, 1),
        "identd": bfa(np.eye(128, dtype=f32)),
    }


def kernel(**inputs):
    from concourse.bass_utils import run_bass_kernel_spmd

    if "nc" not in _CACHE:
        _CACHE["nc"] = _build()
    nc = _CACHE["nc"]

    in_maps = [_prep_core(inputs, c) for c in range(NCORES)]
    res = run_bass_kernel_spmd(nc, in_maps, list(range(NCORES)))
    out = np.empty((BFULL, T, NCLS), np.float32)
    for c in range(NCORES):
        out[c * B:(c + 1) * B] = (res.results[c]["probsT"]
                                  .reshape(NCLS, T, B).transpose(2, 1, 0))
    return out


if __name__ == "__main__":
    _build()
    print("build ok")



# revision 2
# speedup vs baseline: 812.9127x; 812.9127x over previous
"""Trainium2 Bass kernel for nn_Attention_90658169684243.

Attention-LSTM decoder: 3x3 conv (512->512) over [B,512,8,32] feature maps,
26 sequential steps of {additive attention over 256 spatial positions,
2-layer LSTM}, and a linear head.

Sharding: data-parallel over batch across 8 cores (B=256 -> 32/core), all
parameters replicated. bf16 on the matmul path with fp32 PSUM accumulation.

Key structure:
- q computed directly in transposed layout via host-fused W2 = w1x1 @ h2h
  (v never materialized; bh_proj/h2h_b/conv_h2h_b folded into bias2T).
- attention e-matmuls use per-pair masked wsc lhsT columns so each batch
  row lands on its own PSUM partition (two batches per matmul, the "wrong"
  half of each row killed by a rank-2 -30 mask matmul before softmax).
- context matmuls use a diagonalized alpha lhsT written in-place by DVE
  block transposes with a stride-34 AP; no PSUM row extraction anywhere.
- LSTM2 input path fused host-side: Wfu2 = rnn2_w_ih @ hlin_w (cur never
  materialized); gates reordered (i,f,o,g) so the three sigmoids take one
  activation + one affine op.
- LSTM1 recurrent + onehot matmuls issued early (before attention) so PE
  fills its tanh-phase idle slots.
"""

import numpy as np
import ml_dtypes

bfnp = ml_dtypes.bfloat16

NCORES = 8
BFULL = 256
B = BFULL // NCORES   # 32 per core
C = 512
HF, WF = 8, 32
HW = HF * WF          # 256
T = 26
HS = 512
NCLS = 38
G4 = 4 * HS           # 2048

_CACHE = {}


def _build():
    import contextlib

    import concourse.bacc as bacc
    import concourse.mybir as mybir
    from concourse import tile
    from concourse.ap import AP

    dt = mybir.dt
    f32 = dt.float32
    bf = dt.bfloat16
    AF = mybir.ActivationFunctionType
    OP = mybir.AluOpType

    nc = bacc.Bacc(None)

    def din(name, shape, dtype=bf):
        return nc.dram_tensor(name, shape, dtype, kind="ExternalInput")

    fm_ci = din("fm_ci", [4, 128, B, HF, WF])
    w9d = din("w9d", [3, 3, 4, 128, C])
    conv_bT = din("conv_bT", [4, 128, 1], f32)
    h0T = din("h0T", [4, 128, B])
    c0 = din("c0", [B, HS], f32)
    onehT = din("onehT", [NCLS + 1, T, B])
    W2fd = din("W2fd", [4, 128, HS])
    bias2Td = din("bias2Td", [128, 4, B], f32)
    wsc_paird = din("wsc_paird", [4, 128, 16, 8])
    maskAd = din("maskAd", [2, B])
    maskBd = din("maskBd", [2, 2 * HW])
    wih1Td = din("wih1Td", [4, 128, G4])
    tail1Td = din("tail1Td", [NCLS + 1, G4])
    whh1Td = din("whh1Td", [4, 128, G4])
    wfu2Td = din("wfu2Td", [4, 128, G4])
    whh2Td = din("whh2Td", [4, 128, G4])
    b2row = din("b2row", [1, G4])
    gen_wTd = din("gen_wTd", [4, 128, NCLS])
    gen_bTd = din("gen_bTd", [NCLS, 1], f32)
    identd = din("identd", [128, 128])

    probsT = nc.dram_tensor("probsT", [NCLS, T * B], f32, kind="ExternalOutput")

    with tile.TileContext(nc) as tc:
        stack = contextlib.ExitStack()
        const = stack.enter_context(tc.tile_pool(name="const", bufs=1))
        big = stack.enter_context(tc.tile_pool(name="big", bufs=1))
        state = stack.enter_context(tc.tile_pool(name="state", bufs=2))

        fmh = [big.tile([128, B, HW], bf, tag=f"fmh{i}", name=f"fmh{i}")
               for i in range(4)]
        fmhT = [big.tile([128, B, C], bf, tag=f"fmhT{i}", name=f"fmhT{i}")
                for i in range(2)]

        def cload(name, src, shape, dtype=bf, pool=None):
            t = (pool or const).tile(shape, dtype, tag=name, name=name)
            nc.sync.dma_start(t[:], src)
            return t

        ones = const.tile([1, B], bf, tag="ones", name="ones")
        nc.vector.memset(ones[:], 1.0)

        # ---------------- phase 1: conv ----------------
        with (
            tc.tile_pool(name="cpad", bufs=1) as cpad,
            tc.tile_pool(name="cw", bufs=1) as cw,
            tc.tile_pool(name="cps", bufs=4, space="PSUM") as cps,
            tc.tile_pool(name="cpt", bufs=4, space="PSUM") as cpt,
        ):
            ident = cw.tile([128, 128], bf, tag="ident", name="ident")
            nc.sync.dma_start(ident[:], identd[:])
            conv_b = []
            for k in range(4):
                cb = cw.tile([128, 1], f32, tag=f"conv_b{k}", name=f"conv_b{k}")
                nc.sync.dma_start(cb[:], conv_bT[k])
                conv_b.append(cb)
            w9 = [[[cw.tile([128, C], bf, tag=f"w9_{kh}{kw}{ci}",
                            name=f"w9_{kh}{kw}{ci}")
                    for ci in range(4)] for kw in range(3)] for kh in range(3)]
            for kh in range(3):
                for kw in range(3):
                    for ci in range(4):
                        nc.gpsimd.dma_start(w9[kh][kw][ci][:], w9d[kh, kw, ci])

            BC = 2  # batch chunk for conv
            for bc in range(B // BC):
                b0 = bc * BC
                pads = []
                for ci in range(4):
                    pad = cpad.tile([128, BC, HF + 2, WF + 2], bf,
                                    tag=f"pad{ci}", name=f"pad{ci}")
                    nc.vector.memset(pad[:, :, 0, :], 0.0)
                    nc.vector.memset(pad[:, :, HF + 1, :], 0.0)
                    nc.vector.memset(pad[:, :, 1:HF + 1, 0], 0.0)
                    nc.vector.memset(pad[:, :, 1:HF + 1, WF + 1], 0.0)
                    for b in range(BC):
                        nc.gpsimd.dma_start(pad[:, b, 1:HF + 1, 1:WF + 1],
                                            fm_ci[ci, :, b0 + b])
                    pads.append(pad)
                for co in range(4):
                    ps = cps.tile([128, BC, HW], f32, tag="pscv", name="pscv")
                    idx = 0
                    for kh in range(3):
                        for kw in range(3):
                            for ci in range(4):
                                nc.tensor.matmul(
                                    ps[:],
                                    w9[kh][kw][ci][:, co * 128:(co + 1) * 128],
                                    pads[ci][:, :, kh:kh + HF, kw:kw + WF],
                                    start=(idx == 0), stop=(idx == 35))
                                idx += 1
                    for b in range(BC):
                        nc.vector.tensor_scalar_add(
                            fmh[co][:, b0 + b, :], ps[:, b, :],
                            conv_b[co][:, 0:1])
                    for b in range(BC):
                        for hh in range(2):
                            pt = cpt.tile([128, 128], bf, tag="pst", name="pst")
                            nc.tensor.transpose(
                                pt[:],
                                fmh[co][:, b0 + b, hh * 128:(hh + 1) * 128],
                                ident[:])
                            nc.vector.tensor_copy(
                                fmhT[hh][:, b0 + b, co * 128:(co + 1) * 128],
                                pt[:])

        # ---------------- phase 2: 26-step scan ----------------
        wconst = stack.enter_context(tc.tile_pool(name="wconst", bufs=1))
        W2f = [cload(f"W2f{k}", W2fd[k], [128, HS], pool=wconst) for k in range(4)]
        bias2T4 = cload("bias2T4", bias2Td[:], [128, 4, B], f32, pool=wconst)
        wsc_pair = [cload(f"wscp{k}", wsc_paird[k], [128, 16, 8], pool=wconst)
                    for k in range(4)]
        ident8 = cload("ident8", identd[0:8, 0:8], [8, 8], pool=wconst)
        maskA = cload("maskA", maskAd[:], [2, B], pool=wconst)
        maskB = cload("maskB", maskBd[:], [2, 2 * HW], pool=wconst)
        h1T = [cload(f"h1T_{k}", h0T[k], [128, B], pool=wconst) for k in range(4)]
        h2T = [cload(f"h2T_{k}", h0T[k], [128, B], pool=wconst) for k in range(4)]
        c1 = cload("c1", c0[:], [B, HS], f32, pool=wconst)
        c2 = cload("c2", c0[:], [B, HS], f32, pool=wconst)
        tail1T = cload("tail1T", tail1Td[:], [NCLS + 1, G4], pool=wconst)
        b2r = cload("b2r", b2row[:], [1, G4], pool=wconst)
        gen_wT = [cload(f"gen_wT{k}", gen_wTd[k], [128, NCLS], pool=wconst)
                  for k in range(4)]
        gen_bT = cload("gen_bT", gen_bTd[:], [NCLS, 1], f32, pool=wconst)
        oneh = cload("oneh", onehT[:], [NCLS + 1, T, B], pool=wconst)
        h2all = [big.tile([128, T * B], bf, tag=f"h2all{i}", name=f"h2all{i}")
                 for i in range(4)]
        # diagonalized alpha lhsT tiles: D[m][p, b, r] nonzero only at r == b,
        # written each step via stride-34 transpose; everything else stays 0.
        Dg = [big.tile([128, B, 33], bf, tag=f"Dg{m}", name=f"Dg{m}")
              for m in range(4)]
        for m in range(4):
            nc.vector.memset(Dg[m][:], 0.0)

        sb = stack.enter_context(tc.tile_pool(name="sb", bufs=2))
        sb1 = stack.enter_context(tc.tile_pool(name="sb1", bufs=1))
        tp = stack.enter_context(tc.tile_pool(name="tp", bufs=2))
        ws = stack.enter_context(tc.tile_pool(name="ws", bufs=2))
        ws3 = stack.enter_context(tc.tile_pool(name="ws3", bufs=3))
        mm = stack.enter_context(tc.tile_pool(name="mm", bufs=1, space="PSUM"))
        mq = stack.enter_context(tc.tile_pool(name="mq", bufs=1, space="PSUM"))
        pe8 = stack.enter_context(tc.tile_pool(name="pe8", bufs=2, space="PSUM"))
        mg = stack.enter_context(tc.tile_pool(name="mg", bufs=1, space="PSUM"))

        for t in range(T):
            # ---- qT = (h2 @ W2f.T + bias2).T, packed [128, 4, B] ----
            ps_q4 = mq.tile([128, 4, B], f32, tag="mq", name="mq")
            for j in range(4):
                for kt in range(4):
                    nc.tensor.matmul(ps_q4[:, j, :],
                                     W2f[kt][:, j * 128:(j + 1) * 128],
                                     h2T[kt][:], start=(kt == 0), stop=(kt == 3))
            q4 = sb.tile([128, 4, B], f32, tag="q4", name="q4")
            nc.vector.tensor_tensor(q4[:], ps_q4[:], bias2T4[:], OP.add)

            # ---- LSTM1 recurrent + onehot matmuls (fill PE while ACT tanhs) --
            ps_g1 = [mg.tile([B, 2 * HS], f32, tag=f"g{h}", name=f"g{h}")
                     for h in range(2)]
            for h in range(2):
                for k in range(4):
                    for nb in range(2):
                        w = ws.tile([128, HS], bf, tag="wh1", name="wh1")
                        nc.gpsimd.dma_start(
                            w[:], whh1Td[k][:, (2 * h + nb) * HS:
                                            (2 * h + nb + 1) * HS])
                        nc.tensor.matmul(
                            ps_g1[h][:, nb * HS:(nb + 1) * HS], h1T[k][:],
                            w[:], start=(k == 0), stop=False)
                for nb in range(2):
                    g = 2 * h + nb
                    nc.tensor.matmul(ps_g1[h][:, nb * HS:(nb + 1) * HS],
                                     oneh[:, t, :],
                                     tail1T[:, g * HS:(g + 1) * HS],
                                     start=False, stop=False)

            # ---- attention, pipelined per octet of 8 batches: e-matmuls,
            # softmax, alpha transpose-to-diag, and ctx matmuls all slot in
            # behind the ACT tanh stream. ctx for octet oc is emitted after
            # the e-matmuls of octet oc+1 so exp(oc+1) is never queued
            # behind ctx work on PE.
            ps_c = mm.tile([B, HS], f32, tag="mm", name="mm")
            alphas = [None] * 4

            def emit_ctx(oc):
                for c in range(4):
                    pT = mq.tile([128, 8], bf, tag="mq", name="pT")
                    nc.tensor.transpose(pT[:],
                                        alphas[oc][:, c * 128:(c + 1) * 128],
                                        ident8[:])
                    base = Dg[c][:]
                    nc.vector.tensor_copy(
                        AP(base.tensor, base.offset + oc * 8 * 34,
                           [list(base.ap[0]), [34, 8]]),
                        pT[:])
                for m in range(4):
                    for i in range(m // 2, 8, 2):
                        b = oc * 8 + i
                        nc.tensor.matmul(
                            ps_c[:], Dg[m][:, b, 0:32], fmhT[m % 2][:, b, :],
                            start=(oc == 0 and m == 0 and i == 0),
                            stop=(oc == 3 and m == 3 and i == 7))

            for oc in range(4):
                ps_e = pe8.tile([8, 2 * HW], f32, tag="e8", name="e8")
                nc.tensor.matmul(ps_e[:], maskA[:, oc * 8:(oc + 1) * 8],
                                 maskB[:], start=True, stop=False)
                for ct in range(4):
                    tt = tp.tile([128, 8, HW], bf, tag="t", name="t")
                    for i in range(8):
                        b = oc * 8 + i
                        nc.vector.tensor_scalar_add(
                            tt[:, i, :], fmh[ct][:, b, :], q4[:, ct, b:b + 1])
                    nc.scalar.activation(tt[:], tt[:], AF.Tanh)
                    for pp in range(4):
                        pair = oc * 4 + pp
                        nc.tensor.matmul(
                            ps_e[:], wsc_pair[ct][:, pair, :],
                            tt[:, pp * 2:pp * 2 + 2, :],
                            start=False, stop=(ct == 3 and pp == 3))
                if oc > 0:
                    emit_ctx(oc - 1)
                # softmax over the correct hw block (no max: |e| is small);
                # the masked wrong block underflows to ~0.
                expz = sb1.tile([8, 2 * HW], f32, tag="expz", name="expz")
                zsum = sb.tile([8, 2], f32, tag="zsum", name="zsum")
                nc.scalar.activation(expz[:], ps_e[:], AF.Exp,
                                     accum_out=zsum[:, 0:1])
                nc.vector.reciprocal(zsum[:, 1:2], zsum[:, 0:1])
                alphas[oc] = sb.tile([8, 2 * HW], bf, tag="alpha8",
                                     name="alpha8")
                nc.vector.tensor_scalar_mul(alphas[oc][:], expz[:],
                                            zsum[:, 1:2])
            emit_ctx(3)
            ctx_bf = sb1.tile([B, HS], bf, tag="ctx", name="ctx_bf")
            nc.vector.tensor_copy(ctx_bf[:], ps_c[:])
            xT = [sb.tile([128, B], bf, tag=f"xT{k}", name=f"xT{k}")
                  for k in range(4)]
            t32(nc, xT, ctx_bf[:], HS)

            # ---- LSTM1 input matmuls (IF chunk completes first) ----
            for h in range(2):
                for k in range(4):
                    for nb in range(2):
                        w = ws3.tile([128, HS], bf, tag="wi1", name="wi1")
                        nc.sync.dma_start(
                            w[:], wih1Td[k][:, (2 * h + nb) * HS:
                                            (2 * h + nb + 1) * HS])
                        nc.tensor.matmul(
                            ps_g1[h][:, nb * HS:(nb + 1) * HS], xT[k][:],
                            w[:], start=False, stop=(k == 3))

            def lstm_cell(ps_if, ps_og, c_prev, tag):
                # gate order i, f | o, g in two psum chunks: the (i,f) chunk
                # activates while PE still fills the (o,g) chunk.
                th = sb1.tile([B, 4, HS], f32, tag="th4", name="th4")
                nc.scalar.activation(th[:, 0:2, :], ps_if[:], AF.Tanh,
                                     scale=0.5)
                nc.scalar.activation(th[:, 3, :], ps_og[:, HS:2 * HS], AF.Tanh)
                nc.scalar.activation(th[:, 2, :], ps_og[:, 0:HS], AF.Tanh,
                                     scale=0.5)
                nc.vector.tensor_scalar(th[:, 0:2, :], th[:, 0:2, :],
                                        0.5, 0.5, OP.mult, OP.add)
                nc.vector.tensor_tensor(th[:, 1, :], th[:, 1, :], c_prev[:],
                                        OP.mult)
                nc.vector.tensor_tensor(th[:, 0, :], th[:, 0, :], th[:, 3, :],
                                        OP.mult)
                c_new = state.tile([B, HS], f32, tag=f"c{tag}", name=f"c{tag}")
                nc.vector.tensor_tensor(c_new[:], th[:, 1, :], th[:, 0, :],
                                        OP.add)
                nc.vector.tensor_scalar(th[:, 2, :], th[:, 2, :],
                                        0.5, 0.5, OP.mult, OP.add)
                nc.scalar.activation(th[:, 3, :], c_new[:], AF.Tanh)
                h_bf = sb.tile([B, HS], bf, tag="hbf", name=f"hbf{tag}")
                nc.vector.tensor_tensor(h_bf[:], th[:, 2, :], th[:, 3, :],
                                        OP.mult)
                return c_new, h_bf

            c1, h1_bf = lstm_cell(ps_g1[0], ps_g1[1], c1, "1")
            h1T = [state.tile([128, B], bf, tag=f"h1T{k}", name=f"h1T{k}")
                   for k in range(4)]
            t32(nc, h1T, h1_bf[:], HS)

            # ---- LSTM2 gates (hlin fused into wfu2); whh2+b2 parts only ----
            # need h2(t-1), so they run under cell1's tail.
            ps_g2 = [mg.tile([B, 2 * HS], f32, tag=f"g{h}", name=f"g{h}")
                     for h in range(2)]
            for h in range(2):
                for k in range(4):
                    for nb in range(2):
                        w = ws.tile([128, HS], bf, tag="wh2", name="wh2")
                        nc.gpsimd.dma_start(
                            w[:], whh2Td[k][:, (2 * h + nb) * HS:
                                            (2 * h + nb + 1) * HS])
                        nc.tensor.matmul(
                            ps_g2[h][:, nb * HS:(nb + 1) * HS], h2T[k][:],
                            w[:], start=(k == 0), stop=False)
                for nb in range(2):
                    g = 2 * h + nb
                    nc.tensor.matmul(ps_g2[h][:, nb * HS:(nb + 1) * HS],
                                     ones[:], b2r[:, g * HS:(g + 1) * HS],
                                     start=False, stop=False)
            for h in range(2):
                for k in range(4):
                    for nb in range(2):
                        w = ws3.tile([128, HS], bf, tag="wf2", name="wf2")
                        nc.sync.dma_start(
                            w[:], wfu2Td[k][:, (2 * h + nb) * HS:
                                            (2 * h + nb + 1) * HS])
                        nc.tensor.matmul(
                            ps_g2[h][:, nb * HS:(nb + 1) * HS], h1T[k][:],
                            w[:], start=False, stop=(k == 3))

            c2, h2_bf = lstm_cell(ps_g2[0], ps_g2[1], c2, "2")
            h2T = [h2all[k][:, t * B:(t + 1) * B] for k in range(4)]
            t32(nc, h2T, h2_bf[:], HS)

        # ---------------- phase 3: probs = h2_all @ gen_w^T + gen_b ---------
        out_sb = sb1.tile([NCLS, T * B], f32, tag="th4", name="out_sb")
        for n0, n1 in ((0, 512), (512, T * B)):
            ps_p = mm.tile([NCLS, n1 - n0], f32, tag="mm", name="mm")
            for k in range(4):
                nc.tensor.matmul(ps_p[:], gen_wT[k][:], h2all[k][:, n0:n1],
                                 start=(k == 0), stop=(k == 3))
            nc.scalar.activation(out_sb[:, n0:n1], ps_p[:], AF.Identity,
                                 bias=gen_bT[:, 0:1])
        nc.sync.dma_start(probsT[:], out_sb[:])

        stack.close()

    nc.compile()
    return nc


def t32(nc, dst_tiles, src_ap, ncols):
    """Transpose src [32, ncols] into tiles of [128, 32] via DVE 32x32 block
    transposes: block j of src lands at dst_tiles[j // 4] rows (j % 4)*32."""
    for j in range(ncols // 32):
        kt, r = j // 4, (j % 4) * 32
        nc.vector.transpose(dst_tiles[kt][r:r + 32, :],
                            src_ap[:, j * 32:(j + 1) * 32])


# gate reorder: pytorch (i, f, g, o) -> kernel (i, f, o, g)
_PERM = np.r_[0:1024, 1536:2048, 1024:1536]


def _prep_core(inputs, c):
    """Per-core input map (host-side reshape/transpose/cast only)."""
    f32 = np.float32
    sl = slice(c * B, (c + 1) * B)
    fm = np.asarray(inputs["feature_map"], f32)[sl]
    fm_ci = np.ascontiguousarray(fm.transpose(1, 0, 2, 3)).reshape(
        4, 128, B, HF, WF)

    def bfa(x):
        return np.ascontiguousarray(x).astype(bfnp)

    w9 = np.asarray(inputs["conv_m2h_w"], f32).transpose(2, 3, 1, 0)
    w9d = w9.reshape(3, 3, 4, 128, C)

    hh = np.asarray(inputs["hidden_h"], f32)
    hc = np.asarray(inputs["hidden_c"], f32)
    h0 = ((hh[0] + hh[1]) * 0.5)[sl]
    c0 = ((hc[0] + hc[1]) * 0.5)[sl]

    text = np.asarray(inputs["text"])[sl]
    onehT = np.zeros((NCLS + 1, T, B), f32)
    for b in range(B):
        for t in range(T):
            onehT[int(text[b, t]), t, b] = 1.0
    onehT[NCLS] = 1.0

    # fused q path: q = bh_pre @ w1x1.T + h2 @ (w1x1 @ h2h).T
    h2h_w = np.asarray(inputs["h2h_w"], f32)
    w1x1 = np.asarray(inputs["conv_h2h_w"], f32)[:, :, 0, 0]
    bh_proj = (np.asarray(inputs["batch_H"], f32)[sl].mean(axis=1)
               @ np.asarray(inputs["i2h_w"], f32).T)
    bias2 = ((bh_proj + np.asarray(inputs["h2h_b"], f32)[None]) @ w1x1.T
             + np.asarray(inputs["conv_h2h_b"], f32)[None])
    W2 = w1x1 @ h2h_w

    wsc = np.asarray(inputs["score_w"], f32)[0, :, 0, 0]
    wsc_pair = np.zeros((4, 128, 16, 8), f32)
    wt = wsc.reshape(4, 128)
    for p in range(16):
        wsc_pair[:, :, p, 2 * (p % 4)] = wt
        wsc_pair[:, :, p, 2 * (p % 4) + 1] = wt

    maskA = np.zeros((2, B), f32)
    maskA[0, 0::2] = 1.0
    maskA[1, 1::2] = 1.0
    maskB = np.zeros((2, 2 * HW), f32)
    maskB[0, HW:] = -30.0
    maskB[1, :HW] = -30.0

    b1 = np.asarray(inputs["rnn1_b_ih"], f32) + np.asarray(inputs["rnn1_b_hh"], f32)
    wih1T = np.asarray(inputs["rnn1_w_ih"], f32).T[:, _PERM]
    tail1T = np.concatenate([wih1T[512:550], b1[_PERM][None]], axis=0)

    hlin_w = np.asarray(inputs["hlin_w"], f32)
    hlin_b = np.asarray(inputs["hlin_b"], f32)
    w2ih = np.asarray(inputs["rnn2_w_ih"], f32)
    wfu2 = w2ih @ hlin_w
    b2 = (np.asarray(inputs["rnn2_b_ih"], f32)
          + np.asarray(inputs["rnn2_b_hh"], f32) + w2ih @ hlin_b)

    return {
        "fm_ci": bfa(fm_ci),
        "w9d": bfa(w9d),
        "conv_bT": np.asarray(inputs["conv_m2h_b"], f32).reshape(4, 128, 1),
        "h0T": bfa(h0.T.reshape(4, 128, B)),
        "c0": np.ascontiguousarray(c0),
        "onehT": bfa(onehT),
        "W2fd": bfa(W2.T.reshape(4, 128, HS)),
        "bias2Td": np.ascontiguousarray(
            bias2.T.reshape(4, 128, B).transpose(1, 0, 2)),
        "wsc_paird": bfa(wsc_pair),
        "maskAd": bfa(maskA),
        "maskBd": bfa(maskB),
        "wih1Td": bfa(wih1T[:512].reshape(4, 128, G4)),
        "tail1Td": bfa(tail1T),
        "whh1Td": bfa(np.asarray(inputs["rnn1_w_hh"], f32).T[:, _PERM]
                      .reshape(4, 128, G4)),
        "wfu2Td": bfa(wfu2.T[:, _PERM].reshape(4, 128, G4)),
        "whh2Td": bfa(np.asarray(inputs["rnn2_w_hh"], f32).T[:, _PERM]
                      .reshape(4, 128, G4)),
        "b2row": bfa(b2[_PERM][None]),
        "gen_wTd": bfa(np.asarray(inputs["gen_w"], f32).T.reshape(4, 128, NCLS)),
        "gen_bTd": np.asarray(inputs["gen_b"], f32).reshape(NCLS, 1),
        "identd": bfa(np.eye(128, dtype=f32)),
    }


def kernel(**inputs):
    from concourse.bass_utils import run_bass_kernel_spmd

    if "nc" not in _CACHE:
        _CACHE["nc"] = _build()
    nc = _CACHE["nc"]

    in_maps = [_prep_core(inputs, c) for c in range(NCORES)]
    res = run_bass_kernel_spmd(nc, in_maps, list(range(NCORES)))
    out = np.empty((BFULL, T, NCLS), np.float32)
    for c in range(NCORES):
        out[c * B:(c + 1) * B] = (res.results[c]["probsT"]
                                  .reshape(NCLS, T, B).transpose(2, 1, 0))
    return out


if __name__ == "__main__":
    _build()
    print("build ok")
